# revision 1
# baseline (speedup 1.0000x reference)
"""3-layer GATv2 (heads=1, eval) on 8 Trainium2 NeuronCores — Bass/Tile.

kernel(**inputs) takes the FULL inputs (x [100000,128] f32, Wl/Wr [3,128,128],
att [3,128], b [3,128], edge_index [2,1600000] int64) and returns the FULL
[100000, 128] float32 output.

Strategy (graph/data parallel, per the node-partition sharding):
  * core c owns dst nodes [c*12500, (c+1)*12500); edges are grouped on the
    host by (dst block of 128 nodes, src bucket of 25000 rows) with a uniform
    per-(block,bucket) slot budget (multiple of 128) so one SPMD program
    serves all 8 cores; pad slots use idx 0 and a dloc sentinel of 512 that
    zeroes their one-hot rows downstream.
  * per layer, tables XL = h@Wl (all N rows, bf16) and XR = h@Wr (local rows)
    live in DRAM; XL is built from local rows and AllGather'd. Edge features
    are fetched with SWDGE dma_gather (int16 indices, 256B rows).
  * scores: v = xl[src]+xr[dst]; z = max(v, 0.2*v); e = sum_d z*att; w=exp(e)
    (no segment-max: |e| stays < ~30 for this model). Softmax-weighted
    aggregation per 128-dst block via PE: one-hot weight matrices
    W_s[e,n] = w_e * [dloc_e == n] accumulate num = sum W_s.T @ xl_s and
    den = sum W_s.T @ 1 in PSUM; out = num/(den+1e-16) + bias (+relu).
  * the next layer's XL/XR rows are produced in the same block pass
    (transpose via PE, two 128x128 matmuls), so only the AllGather separates
    layers.
"""

import os
from contextlib import ExitStack

import numpy as np
import ml_dtypes

import concourse.bacc as bacc
import concourse.mybir as mybir
import concourse.tile as tile
from concourse._compat import cdiv
from concourse.masks import make_identity
from concourse.bass_utils import run_bass_kernel_spmd

F32 = mybir.dt.float32
BF16 = mybir.dt.bfloat16
I16 = mybir.dt.int16
AX = mybir.AxisListType
OP = mybir.AluOpType
ACTF = mybir.ActivationFunctionType

D = 128
P = 128


class Cfg:
    def __init__(self, N, cores, bucket, b1, sb):
        assert N % cores == 0
        self.N, self.CORES = N, cores
        self.NPC = N // cores
        self.NBLK = cdiv(self.NPC, P)
        self.LASTW = self.NPC - (self.NBLK - 1) * P
        self.BUCKET = bucket
        self.NBUCK = cdiv(N, bucket)
        assert b1 % P == 0
        self.B1 = b1
        self.SLOTS = self.NBUCK * b1
        self.S = self.SLOTS // P
        self.SB = sb
        self.NSB = cdiv(self.NBLK, sb)
        self.IDXCOLS_TOT = sum(
            self.sbn(g) * self.B1 // 16 * self.NBUCK for g in range(self.NSB)
        )

    def sbn(self, g):
        return min(self.SB, self.NBLK - g * self.SB)


def _wrap16(v):
    L = v.size
    assert L % 16 == 0
    w = v.reshape(L // 16, 16).T.astype(np.int16)
    return np.tile(w, (8, 1))


def host_prep(cfg, edge_index):
    src = np.asarray(edge_index[0], dtype=np.int64)
    dst = np.asarray(edge_index[1], dtype=np.int64)
    cores = []
    for c in range(cfg.CORES):
        base = c * cfg.NPC
        m = (dst >= base) & (dst < base + cfg.NPC)
        es, ed = src[m], dst[m] - base
        blk = ed // P
        buck = es // cfg.BUCKET
        order = np.lexsort((es, buck, blk))
        es, ed, blk, buck = es[order], ed[order], blk[order], buck[order]
        key = blk * cfg.NBUCK + buck
        bounds = np.searchsorted(key, np.arange(cfg.NBLK * cfg.NBUCK + 1))
        cnt = np.diff(bounds).reshape(cfg.NBLK, cfg.NBUCK)
        if cnt.max() > cfg.B1:
            raise ValueError(f"bucket overflow: {cnt.max()} > {cfg.B1}")
        xl_slots = np.zeros((cfg.NBLK, cfg.NBUCK, cfg.B1), np.int64)
        xr_slots = np.zeros((cfg.NBLK, cfg.NBUCK, cfg.B1), np.int64)
        dl_slots = np.full((cfg.NBLK, cfg.NBUCK, cfg.B1), 512.0, np.float32)
        for b in range(cfg.NBLK):
            for k in range(cfg.NBUCK):
                i0, i1 = bounds[b * cfg.NBUCK + k], bounds[b * cfg.NBUCK + k + 1]
                n = i1 - i0
                xl_slots[b, k, :n] = es[i0:i1] - k * cfg.BUCKET
                xr_slots[b, k, :n] = ed[i0:i1]
                dl_slots[b, k, :n] = (ed[i0:i1] - b * P).astype(np.float32)
        xl_cols, xr_cols = [], []
        for g in range(cfg.NSB):
            sbn = cfg.sbn(g)
            for k in range(cfg.NBUCK):
                xl_cols.append(
                    _wrap16(xl_slots[g * cfg.SB : g * cfg.SB + sbn, k, :].reshape(-1))
                )
                xr_cols.append(
                    _wrap16(xr_slots[g * cfg.SB : g * cfg.SB + sbn, k, :].reshape(-1))
                )
        xl_idx = np.concatenate(xl_cols, axis=1)
        xr_idx = np.concatenate(xr_cols, axis=1)
        dl = dl_slots.reshape(cfg.NBLK, cfg.S, P)
        dloc = np.ascontiguousarray(dl.transpose(2, 0, 1).reshape(P, cfg.NBLK * cfg.S))
        cores.append(dict(xl_idx=xl_idx, xr_idx=xr_idx, dloc=dloc))
    return cores


def host_consts(cfg, Wl, Wr, att, b, x):
    Wl = np.asarray(Wl, np.float32)
    Wr = np.asarray(Wr, np.float32)
    att = np.asarray(att, np.float32)
    b = np.asarray(b, np.float32)
    x = np.asarray(x, np.float32)
    wl_all = Wl.reshape(3 * D, D).astype(ml_dtypes.bfloat16)
    wr_all = Wr.reshape(3 * D, D).astype(ml_dtypes.bfloat16)
    att_mat = np.concatenate(
        [np.tile(att[l][None, :], (P, 1)) for l in range(3)], 0
    ).astype(ml_dtypes.bfloat16)
    bias_mat = np.concatenate(
        [np.tile(b[l][None, :], (P, 1)) for l in range(3)], 0
    ).astype(np.float32)
    iota = np.tile(np.arange(P, dtype=np.float32)[None, :], (P, 1)).astype(
        ml_dtypes.bfloat16
    )
    out = []
    for c in range(cfg.CORES):
        xT = np.ascontiguousarray(x[c * cfg.NPC : (c + 1) * cfg.NPC].T).astype(
            ml_dtypes.bfloat16
        )
        out.append(
            dict(
                xT_loc=xT,
                Wl_all=wl_all,
                Wr_all=wr_all,
                att_mat=att_mat,
                bias_mat=bias_mat,
                iota_mat=iota,
            )
        )
    return out


def build_program(cfg):
    nc = bacc.Bacc(
        "TRN2", target_bir_lowering=False, debug=False, num_devices=cfg.CORES
    )
    NPC, NBLK, NBUCK, B1, S, SB, NSB = (
        cfg.NPC, cfg.NBLK, cfg.NBUCK, cfg.B1, cfg.S, cfg.SB, cfg.NSB,
    )

    xT_loc = nc.dram_tensor("xT_loc", [P, NPC], BF16, kind="ExternalInput")
    Wl_all = nc.dram_tensor("Wl_all", [3 * D, D], BF16, kind="ExternalInput")
    Wr_all = nc.dram_tensor("Wr_all", [3 * D, D], BF16, kind="ExternalInput")
    att_mat = nc.dram_tensor("att_mat", [3 * P, D], BF16, kind="ExternalInput")
    bias_mat = nc.dram_tensor("bias_mat", [3 * P, D], F32, kind="ExternalInput")
    iota_mat = nc.dram_tensor("iota_mat", [P, P], BF16, kind="ExternalInput")
    xl_idx = nc.dram_tensor("xl_idx", [P, cfg.IDXCOLS_TOT], I16, kind="ExternalInput")
    xr_idx = nc.dram_tensor("xr_idx", [P, cfg.IDXCOLS_TOT], I16, kind="ExternalInput")
    dloc = nc.dram_tensor("dloc", [P, NBLK * S], F32, kind="ExternalInput")
    out_loc = nc.dram_tensor("out_loc", [NPC, D], F32, kind="ExternalOutput")

    XLb = [nc.dram_tensor(f"XLb{l}", [NPC, D], BF16) for l in range(3)]
    XR = [nc.dram_tensor(f"XR{l}", [NPC, D], BF16) for l in range(3)]
    XLf = [nc.dram_tensor(f"XLf{l}", [cfg.N, D], BF16) for l in range(3)]

    with tile.TileContext(nc) as tc, ExitStack() as ctx:
        consts = ctx.enter_context(tc.tile_pool(name="consts", bufs=1))
        gpool = ctx.enter_context(tc.tile_pool(name="gath", bufs=2))
        wrk = ctx.enter_context(tc.tile_pool(name="wrk", bufs=2))
        small = ctx.enter_context(tc.tile_pool(name="small", bufs=3))
        wpool = ctx.enter_context(tc.tile_pool(name="wtile", bufs=4))
        idxp = ctx.enter_context(tc.tile_pool(name="idx", bufs=2))
        psA = ctx.enter_context(tc.tile_pool(name="psA", bufs=2, space="PSUM"))
        psF = ctx.enter_context(tc.tile_pool(name="psF", bufs=2, space="PSUM"))

        iota_t = consts.tile([P, P], BF16, tag="iota")
        nc.sync.dma_start(iota_t[:], iota_mat[:, :])
        ones_t = consts.tile([P, 1], BF16, tag="ones")
        nc.vector.memset(ones_t[:], 1.0)
        ident_t = consts.tile([P, P], BF16, tag="ident")
        make_identity(nc, ident_t[:])
        wl_t, wr_t, at_t, bi_t = [], [], [], []
        for l in range(3):
            w1 = consts.tile([P, D], BF16, tag=f"wl{l}")
            nc.sync.dma_start(w1[:], Wl_all[l * D : (l + 1) * D, :])
            w2 = consts.tile([P, D], BF16, tag=f"wr{l}")
            nc.sync.dma_start(w2[:], Wr_all[l * D : (l + 1) * D, :])
            a1 = consts.tile([P, D], BF16, tag=f"att{l}")
            nc.sync.dma_start(a1[:], att_mat[l * P : (l + 1) * P, :])
            b1t = consts.tile([P, D], F32, tag=f"bias{l}")
            nc.sync.dma_start(b1t[:], bias_mat[l * P : (l + 1) * P, :])
            wl_t.append(w1); wr_t.append(w2); at_t.append(a1); bi_t.append(b1t)

        for cblk in range(NBLK):
            cw = P if cblk < NBLK - 1 else cfg.LASTW
            xTs = wrk.tile([P, P], BF16, tag="xTs")
            nc.sync.dma_start(xTs[:, :cw], xT_loc[:, cblk * P : cblk * P + cw])
            pxl = psF.tile([P, D], F32, tag="fin")
            nc.tensor.matmul(pxl[:cw, :], xTs[:, :cw], wl_t[0][:], start=True, stop=True)
            sxl = small.tile([P, D], BF16, tag="sxl")
            nc.scalar.activation(sxl[:cw, :], pxl[:cw, :], ACTF.Copy)
            nc.sync.dma_start(XLb[0][cblk * P : cblk * P + cw, :], sxl[:cw, :])
            pxr = psF.tile([P, D], F32, tag="fin")
            nc.tensor.matmul(pxr[:cw, :], xTs[:, :cw], wr_t[0][:], start=True, stop=True)
            sxr = small.tile([P, D], BF16, tag="sxr")
            nc.scalar.activation(sxr[:cw, :], pxr[:cw, :], ACTF.Copy)
            nc.sync.dma_start(XR[0][cblk * P : cblk * P + cw, :], sxr[:cw, :])
        nc.gpsimd.collective_compute(
            "AllGather", OP.bypass,
            replica_groups=[list(range(cfg.CORES))],
            ins=[XLb[0].ap().opt()], outs=[XLf[0].ap().opt()],
        )

        for l in range(3):
            goff = 0
            for g in range(NSB):
                sbn = cfg.sbn(g)
                gcols = sbn * B1 // 16
                ixl = idxp.tile([P, NBUCK * gcols], I16, tag="ixl")
                nc.sync.dma_start(ixl[:], xl_idx[:, goff : goff + NBUCK * gcols])
                ixr = idxp.tile([P, NBUCK * gcols], I16, tag="ixr")
                nc.sync.dma_start(ixr[:], xr_idx[:, goff : goff + NBUCK * gcols])
                dlt = small.tile([P, sbn * S], F32, tag="dlt")
                nc.sync.dma_start(dlt[:], dloc[:, g * SB * S : g * SB * S + sbn * S])
                xlg = gpool.tile([P, NBUCK * sbn * B1], BF16, tag="xlg")
                xrg = gpool.tile([P, NBUCK * sbn * B1], BF16, tag="xrg")
                ni = sbn * B1
                for k in range(NBUCK):
                    kb = k * cfg.BUCKET
                    ke = min(kb + cfg.BUCKET, cfg.N)
                    nc.gpsimd.dma_gather(
                        xlg[:, k * ni : (k + 1) * ni].rearrange("p (m x) -> p m x", x=D),
                        XLf[l][kb:ke, :],
                        ixl[:, k * gcols : (k + 1) * gcols],
                        ni, ni, D, single_packet=False,
                    )
                    nc.gpsimd.dma_gather(
                        xrg[:, k * ni : (k + 1) * ni].rearrange("p (m x) -> p m x", x=D),
                        XR[l][:, :],
                        ixr[:, k * gcols : (k + 1) * gcols],
                        ni, ni, D, single_packet=False,
                    )
                nums = psA.tile([P, SB, D], F32, tag="nums")
                dens = psA.tile([P, SB], F32, tag="dens")
                for bl in range(sbn):
                    b = g * SB + bl
                    bw = P if b < NBLK - 1 else cfg.LASTW
                    xlb = xlg[:].rearrange(
                        "p (k s b1) -> p k s b1", k=NBUCK, s=sbn
                    )[:, :, bl, :]
                    xrb = xrg[:].rearrange(
                        "p (k s b1) -> p k s b1", k=NBUCK, s=sbn
                    )[:, :, bl, :]
                    v = wrk.tile([P, NBUCK * B1], BF16, tag="v")
                    nc.vector.tensor_tensor(
                        v[:].rearrange("p (k b1) -> p k b1", k=NBUCK),
                        xlb, xrb, op=OP.add,
                    )
                    t = wrk.tile([P, NBUCK * B1], BF16, tag="t")
                    nc.scalar.activation(t[:], v[:], ACTF.Copy, scale=0.2)
                    nc.vector.tensor_tensor(v[:], v[:], t[:], op=OP.max)
                    nc.vector.tensor_tensor(
                        t[:].rearrange("p (s x) -> p s x", x=D),
                        v[:].rearrange("p (s x) -> p s x", x=D),
                        at_t[l][:].unsqueeze(1).to_broadcast([P, S, D]),
                        op=OP.mult,
                    )
                    sc = small.tile([P, S], F32, tag="sc")
                    nc.vector.tensor_reduce(
                        sc[:], t[:].rearrange("p (s x) -> p s x", x=D),
                        axis=AX.X, op=OP.add,
                    )
                    w = small.tile([P, S], F32, tag="w")
                    nc.scalar.activation(w[:], sc[:], ACTF.Exp)
                    for s in range(S):
                        k, j = divmod(s, B1 // P)
                        Ws = wpool.tile([P, P], BF16, tag="W")
                        nc.vector.tensor_scalar(
                            Ws[:], iota_t[:],
                            dlt[:, bl * S + s : bl * S + s + 1],
                            w[:, s : s + 1],
                            op0=OP.is_equal, op1=OP.mult,
                        )
                        nc.tensor.matmul(
                            nums[:, bl, :], Ws[:],
                            xlb[:, k, j * P : (j + 1) * P],
                            start=(s == 0), stop=(s == S - 1),
                        )
                        nc.tensor.matmul(
                            dens[:, bl : bl + 1], Ws[:], ones_t[:],
                            start=(s == 0), stop=(s == S - 1),
                        )
                    den = small.tile([P, 1], F32, tag="den")
                    nc.vector.tensor_scalar(
                        den[:], dens[:, bl : bl + 1], 1e-16, None, op0=OP.add
                    )
                    rec = small.tile([P, 1], F32, tag="rec")
                    nc.vector.reciprocal(rec[:], den[:])
                    onum = small.tile([P, D], F32, tag="onum")
                    nc.vector.tensor_scalar(
                        onum[:], nums[:, bl, :], rec[:], None, op0=OP.mult
                    )
                    nc.vector.tensor_tensor(onum[:], onum[:], bi_t[l][:], op=OP.add)
                    if l == 2:
                        nc.sync.dma_start(out_loc[b * P : b * P + bw, :], onum[:bw, :])
                    else:
                        hrow = small.tile([P, D], BF16, tag="hrow")
                        nc.scalar.activation(hrow[:], onum[:], ACTF.Relu)
                        pst = psF.tile([P, P], BF16, tag="fint")
                        nc.tensor.transpose(pst[:], hrow[:], ident_t[:])
                        hT = small.tile([P, P], BF16, tag="hT")
                        nc.scalar.activation(hT[:], pst[:], ACTF.Copy)
                        pxl = psF.tile([P, D], F32, tag="fin")
                        nc.tensor.matmul(
                            pxl[:], hT[:], wl_t[l + 1][:], start=True, stop=True
                        )
                        sxl = small.tile([P, D], BF16, tag="sxl")
                        nc.scalar.activation(sxl[:], pxl[:], ACTF.Copy)
                        nc.sync.dma_start(
                            XLb[l + 1][b * P : b * P + bw, :], sxl[:bw, :]
                        )
                        pxr = psF.tile([P, D], F32, tag="fin")
                        nc.tensor.matmul(
                            pxr[:], hT[:], wr_t[l + 1][:], start=True, stop=True
                        )
                        sxr = small.tile([P, D], BF16, tag="sxr")
                        nc.scalar.activation(sxr[:], pxr[:], ACTF.Copy)
                        nc.sync.dma_start(
                            XR[l + 1][b * P : b * P + bw, :], sxr[:bw, :]
                        )
                goff += NBUCK * gcols
            if l < 2:
                nc.gpsimd.collective_compute(
                    "AllGather", OP.bypass,
                    replica_groups=[list(range(cfg.CORES))],
                    ins=[XLb[l + 1].ap().opt()], outs=[XLf[l + 1].ap().opt()],
                )
    nc.compile()
    return nc


def kernel(x, Wl, Wr, att, b, edge_index):
    x = np.asarray(x, np.float32)
    edge_index = np.asarray(edge_index)
    N = x.shape[0]
    CORES = 8

    # uniform slot budget from this input's worst (core, block, bucket)
    bucket = cdiv(N, 4)
    src = np.asarray(edge_index[0], np.int64)
    dst = np.asarray(edge_index[1], np.int64)
    npc = N // CORES
    nblk = cdiv(npc, P)
    mx = 0
    for c in range(CORES):
        m = (dst >= c * npc) & (dst < (c + 1) * npc)
        key = ((dst[m] - c * npc) // P) * 4 + src[m] // bucket
        mx = max(mx, int(np.bincount(key, minlength=nblk * 4).max()))
    b1 = max(cdiv(mx, P) * P, P)

    cfg = Cfg(N=N, cores=CORES, bucket=bucket, b1=b1, sb=4)
    idx_data = host_prep(cfg, edge_index)
    const_data = host_consts(cfg, Wl, Wr, att, b, x)
    nc = build_program(cfg)
    in_maps = [{**idx_data[c], **const_data[c]} for c in range(CORES)]

    prof_dir = os.environ.get("GAT_PROFILE", "")
    if prof_dir:
        import sys
        sys.path.insert(0, "/root/.axon_site")
        from trn_agent_boot import trn_boot
        hook = trn_boot._ntff_profile_via_ctypes("/opt/axon/libaxon_pjrt.so")
        os.makedirs(prof_dir, exist_ok=True)
        with hook(prof_dir, [0]):
            res = run_bass_kernel_spmd(nc, in_maps, core_ids=list(range(CORES)))
    else:
        res = run_bass_kernel_spmd(nc, in_maps, core_ids=list(range(CORES)))

    out = np.concatenate([r["out_loc"] for r in res.results], axis=0)
    return out.astype(np.float32)

